# revision 1
# baseline (speedup 1.0000x reference)
"""Trainium2 Bass kernel for a SwiGLU-style feed-forward block.

reference:
    gate = x @ w1.T ; up = x @ w2.T ; h = silu(gate) * up ; out = h @ w3.T
    x: [4, 2048, 2048] f32, w1/w2: [8192, 2048] f32, w3: [2048, 8192] f32

Strategy: pure data-parallel over the 8192 tokens — each of the 8
NeuronCores gets 1024 tokens and the full weights.  All tensors are
pre-transposed + cast to bf16 on the host so every DMA is contiguous
and the TensorEngine contraction dim always sits on SBUF partitions:

    xT   [E, Ts]  (activations feature-major)
    w1T  [E, H]   w2T [E, H]   w3T [H, E]

Per core, in two token chunks of 512:
    phase A: gateT/upT[h,:] = w1T/w2T-tile^T @ xT-tile  (PSUM, fp32 accum)
             hT[h,:] = silu(gateT) * upT   (ACT Silu + DVE mul, bf16)
    phase B: outT[e,:]  = sum_h w3T-tile^T @ hT-tile    (PSUM accum over H)
Output is outT [E, Ts] fp32 per core; the host reassembles and
transposes back.
"""

import json

import numpy as np
import ml_dtypes

import concourse.bass as bass
import concourse.mybir as mybir
import concourse.tile as tile
from concourse.vector_clock import ScopedClock
from concourse.bass_utils import run_bass_kernel_spmd

# ---------------------------------------------------------------- shapes
N_CORES = 8
EMB = 2048          # E
HID = 8192          # H
T_TOTAL = 8192      # B*S tokens
T_SHARD = T_TOTAL // N_CORES   # 1024 tokens per core
T_CHUNK = 512                  # tokens per on-chip pass
H_BIG = 256                    # w1/w2 slab width in H per DMA
E_SUB = EMB // 128             # 16 contraction subtiles for phase A
H_SUB = HID // 128             # 64 contraction subtiles for phase B

CDT = mybir.dt.bfloat16        # compute dtype on the PE
NP_CDT = ml_dtypes.bfloat16

P = 128
F32 = mybir.dt.float32


class _TileContextSplitWait(tile.TileContext):
    """The walrus build in this environment rejects >1 sync-wait on a
    CTRL (Drain) instruction.  Split the kernel-tail drain's waits into
    single-wait nops emitted just before it."""

    def _drain_and_barrier(self, tick_clock, wait_clock):
        probe = self.nc.sync.nop(nofuse=True)
        wait_clock.add_sem_waits(
            probe.ins, ScopedClock({None: tick_clock.global_clock})
        )
        si = probe.ins.sync_info
        if si is not None and len(si.on_wait) > 1:
            waits = list(si.on_wait)
            probe.ins.sync_info = mybir.SyncInfo(
                on_wait=waits[:1], on_update=list(si.on_update)
            )
            for w in waits[1:]:
                n = self.nc.sync.nop(nofuse=True)
                n.ins.sync_info = mybir.SyncInfo(on_wait=[w], on_update=[])
        self.nc.sync.drain()
        self.nc.all_engine_barrier()
        assert self.sems is not None
        popped = self.nc._tile_sem_poison_stack.pop()
        assert popped is self._sem_poison
        self.nc.clear_and_free_semaphores(list(self.sems.allocated().values()))
        self.nc.all_engine_barrier()


def _split_multi_waits(bir_bytes):
    """The walrus build here accepts at most one sync-wait command per
    instruction (setupSyncWait raises 'Too many sync wait commands').
    Tile attaches however many the dependence analysis needs, so move
    extra waits onto NoOp instructions inserted just before, on the same
    engine's stream — semantically identical, codegen-compatible."""
    bir = json.loads(bir_bytes)
    for fn in bir["functions"]:
        for blk in fn["blocks"]:
            insts = blk.get("instructions")
            if not insts:
                continue
            out = []
            changed = False
            for inst in insts:
                si = inst.get("sync_info")
                waits = (si or {}).get("on_wait") or []
                if len(waits) > 1:
                    changed = True
                    for j, w in enumerate(waits[:-1]):
                        out.append(
                            {
                                "debug": inst.get("debug"),
                                "engine": inst["engine"],
                                "ins": [],
                                "name": f"{inst['name']}-w{j}",
                                "opcode": "NoOp",
                                "outs": [],
                                "sync_info": {"on_update": [], "on_wait": [w]},
                            }
                        )
                    si["on_wait"] = waits[-1:]
                out.append(inst)
            if changed:
                blk["instructions"] = out
    return json.dumps(bir).encode()


def _build_nc():
    nc = bass.Bass(target_bir_lowering=False)

    xt = nc.dram_tensor("xt", [EMB, T_SHARD], CDT, kind="ExternalInput")
    w1t = nc.dram_tensor("w1t", [EMB, HID], CDT, kind="ExternalInput")
    w2t = nc.dram_tensor("w2t", [EMB, HID], CDT, kind="ExternalInput")
    w3t = nc.dram_tensor("w3t", [HID, EMB], CDT, kind="ExternalInput")
    outt = nc.dram_tensor("outt", [EMB, T_SHARD], F32, kind="ExternalOutput")

    # DRAM views with the 128-partition dim innermost on rows
    xt_r = xt[:].rearrange("(s p) t -> p s t", p=P)      # [128, 16, T_SHARD]
    w1t_r = w1t[:].rearrange("(s p) h -> p s h", p=P)    # [128, 16, HID]
    w2t_r = w2t[:].rearrange("(s p) h -> p s h", p=P)
    w3t_r = w3t[:].rearrange("(s p) e -> p s e", p=P)    # [128, 64, EMB]

    n_chunks = T_SHARD // T_CHUNK
    n_hbig = HID // H_BIG
    ht_per_big = H_BIG // P

    with _TileContextSplitWait(nc) as tc:
        with (
            tc.tile_pool(name="xp", bufs=1) as xp,
            tc.tile_pool(name="wp", bufs=3) as wp,
            tc.tile_pool(name="w3p", bufs=2) as w3p,
            tc.tile_pool(name="htp", bufs=1) as htp,
            tc.tile_pool(name="slp", bufs=3) as slp,
            tc.tile_pool(name="op", bufs=3) as op,
            tc.tile_pool(name="ps", bufs=2, space="PSUM") as ps,
        ):
            for c in range(n_chunks):
                t0 = c * T_CHUNK
                if c == 0:
                    # Separate tiles + hand-ordered DMAs for the first
                    # chunk: the first accumulation group only needs
                    # xs0 (512KB) + the first w1 half-slab (512KB), so
                    # issue those two first on the HWDGE FIFO.
                    xparts = [
                        xp.tile([P, 4, T_CHUNK], CDT, name=f"xs{q}")
                        for q in range(4)
                    ]
                    nc.sync.dma_start(
                        xparts[0][:], xt_r[:, 0:4, t0 : t0 + T_CHUNK]
                    )
                    half0 = H_BIG // 2
                    w1a0 = wp.tile([P, E_SUB, half0], CDT, name="w1h0", bufs=1)
                    nc.sync.dma_start(w1a0[:], w1t_r[:, :, 0:half0])
                    for q in range(1, 4):
                        nc.sync.dma_start(
                            xparts[q][:],
                            xt_r[:, 4 * q : 4 * q + 4, t0 : t0 + T_CHUNK],
                        )
                    w1b0 = wp.tile([P, E_SUB, half0], CDT, name="w1h1", bufs=1)
                    nc.sync.dma_start(w1b0[:], w1t_r[:, :, half0:H_BIG])

                    def xview(e):
                        return xparts[e // 4][:, e % 4, :]
                else:
                    xs = xp.tile([P, E_SUB, T_CHUNK], CDT, name="xs")
                    nc.sync.dma_start(xs[:], xt_r[:, :, t0 : t0 + T_CHUNK])

                    def xview(e, xs=xs):
                        return xs[:, e, :]

                ht = htp.tile([P, H_SUB, T_CHUNK], CDT, name="ht")

                # ---------------- phase A: gate/up + silu*up -> hT
                for hb in range(n_hbig):
                    h0 = hb * H_BIG
                    if c == 0 and hb == 0:
                        half = H_BIG // 2

                        def w1view(e, hof, w1a=w1a0, w1b=w1b0, half=half):
                            s = w1a if hof < half else w1b
                            o = hof % half
                            return s[:, e, o : o + P]
                    else:
                        w1s = wp.tile([P, E_SUB, H_BIG], CDT, name="w1s")
                        nc.sync.dma_start(w1s[:], w1t_r[:, :, h0 : h0 + H_BIG])

                        def w1view(e, hof, w1s=w1s):
                            return w1s[:, e, hof : hof + P]

                    w2s = wp.tile([P, E_SUB, H_BIG], CDT, name="w2s")
                    nc.sync.dma_start(w2s[:], w2t_r[:, :, h0 : h0 + H_BIG])

                    def gate_group(htile):
                        hof = htile * P
                        pg = ps.tile([P, T_CHUNK], F32, name="pg")
                        for e in range(E_SUB):
                            nc.tensor.matmul(
                                pg[:],
                                w1view(e, hof),
                                xview(e),
                                start=(e == 0),
                                stop=(e == E_SUB - 1),
                            )
                        return pg

                    def up_group_and_mul(htile, pg):
                        hsub = hb * ht_per_big + htile
                        hof = htile * P
                        pu = ps.tile([P, T_CHUNK], F32, name="pu")
                        for e in range(E_SUB):
                            nc.tensor.matmul(
                                pu[:],
                                w2s[:, e, hof : hof + P],
                                xview(e),
                                start=(e == 0),
                                stop=(e == E_SUB - 1),
                            )
                        sl = slp.tile([P, T_CHUNK], CDT, name="sl")
                        nc.scalar.activation(
                            sl[:], pg[:], mybir.ActivationFunctionType.Silu
                        )
                        nc.vector.tensor_mul(ht[:, hsub, :], sl[:], pu[:])

                    if c == 0 and hb == 0:
                        # w2s lands last during startup: run both gate
                        # groups first so the PE has work while it loads.
                        pgs = [gate_group(ht_) for ht_ in range(ht_per_big)]
                        for ht_, pg in enumerate(pgs):
                            up_group_and_mul(ht_, pg)
                    else:
                        for ht_ in range(ht_per_big):
                            pg = gate_group(ht_)
                            up_group_and_mul(ht_, pg)

                # ---------------- phase B: outT = sum_h w3T^T @ hT
                hh = H_SUB // 2
                for et in range(E_SUB):
                    e0 = et * P
                    # two half-slabs: accumulation can start when the
                    # first 1MB lands instead of waiting for all 2MB
                    w3a = w3p.tile([P, hh, P], CDT, name="w3a")
                    nc.sync.dma_start(w3a[:], w3t_r[:, :hh, e0 : e0 + P])
                    w3b = w3p.tile([P, hh, P], CDT, name="w3b")
                    nc.sync.dma_start(w3b[:], w3t_r[:, hh:, e0 : e0 + P])
                    po = ps.tile([P, T_CHUNK], F32, name="po")
                    for h in range(H_SUB):
                        w3v = w3a[:, h, :] if h < hh else w3b[:, h - hh, :]
                        nc.tensor.matmul(
                            po[:],
                            w3v,
                            ht[:, h, :],
                            start=(h == 0),
                            stop=(h == H_SUB - 1),
                        )
                    ot = op.tile([P, T_CHUNK], F32, name="ot")
                    nc.vector.tensor_copy(ot[:], po[:])
                    nc.sync.dma_start(
                        outt[e0 : e0 + P, t0 : t0 + T_CHUNK], ot[:]
                    )

    fixed = _split_multi_waits(bass.Bass.to_json_bytes(nc))
    nc.to_json_bytes = lambda: fixed
    return nc


_nc_cache = None


def _get_nc():
    global _nc_cache
    if _nc_cache is None:
        _nc_cache = _build_nc()
    return _nc_cache


def _prep_inputs(x, w1, w2, w3):
    xt = np.ascontiguousarray(
        x.reshape(T_TOTAL, EMB).T.astype(NP_CDT)
    )  # [E, T_total]
    w1t = np.ascontiguousarray(w1.T.astype(NP_CDT))  # [E, H]
    w2t = np.ascontiguousarray(w2.T.astype(NP_CDT))  # [E, H]
    w3t = np.ascontiguousarray(w3.T.astype(NP_CDT))  # [H, E]
    in_maps = []
    for i in range(N_CORES):
        sh = np.ascontiguousarray(xt[:, i * T_SHARD : (i + 1) * T_SHARD])
        in_maps.append({"xt": sh, "w1t": w1t, "w2t": w2t, "w3t": w3t})
    return in_maps


def kernel(x, w1, w2, w3, scale_x=None, _trace=False):
    x = np.asarray(x, np.float32)
    w1 = np.asarray(w1, np.float32)
    w2 = np.asarray(w2, np.float32)
    w3 = np.asarray(w3, np.float32)

    nc = _get_nc()
    in_maps = _prep_inputs(x, w1, w2, w3)
    res = run_bass_kernel_spmd(nc, in_maps, list(range(N_CORES)), trace=_trace)

    outt = np.concatenate(
        [np.asarray(res.results[i]["outt"]) for i in range(N_CORES)], axis=1
    )  # [E, T_total]
    out = np.ascontiguousarray(outt.T).reshape(4, 2048, EMB).astype(np.float32)
    if _trace:
        kernel.last_results = res
    return out


if __name__ == "__main__":
    rng = np.random.default_rng(0)
    x = rng.standard_normal((4, 2048, EMB), dtype=np.float32)
    w1 = (rng.standard_normal((HID, EMB), dtype=np.float32) * 0.03).astype(
        np.float32
    )
    w2 = (rng.standard_normal((HID, EMB), dtype=np.float32) * 0.03).astype(
        np.float32
    )
    w3 = (rng.standard_normal((EMB, HID), dtype=np.float32) * 0.015).astype(
        np.float32
    )
    out = kernel(x, w1, w2, w3)
    print("out", out.shape, out.dtype, float(np.abs(out).mean()))



# revision 3
# speedup vs baseline: 1.0334x; 1.0334x over previous
"""Trainium2 Bass kernel for a SwiGLU-style feed-forward block.

reference:
    gate = x @ w1.T ; up = x @ w2.T ; h = silu(gate) * up ; out = h @ w3.T
    x: [4, 2048, 2048] f32, w1/w2: [8192, 2048] f32, w3: [2048, 8192] f32

Strategy: pure data-parallel over the 8192 tokens -- each of the 8
NeuronCores gets 1024 tokens and the full weights, processed in two
512-token chunks.  All tensors are pre-transposed on the host into
partition-major slabs so every DMA is 4-16 KB contiguous per partition:

    xd   [2, 128, 16, 512]      activations, feature-major, per chunk
    w1d  [64, 128, 16, 128]     one slab per 128-wide h-tile
    w2d  [64, 128, 16, 128]     scaled x4 (lossless in bf16)
    w3d  [16, 2, 128, 26, 128]  bf16 down-proj, hsub 0..51, scaled x128
    w38d [16, 128, 6, 2, 128]   fp8e4 down-proj pairs, hsub 52..63, x128

Per chunk:
    phase A: gateT/upT[h,:] = w1/w2-slab^T @ x      (PSUM, fp32)
             sl = silu(gate) [ACT];  ht = sl * pu   [DVE]
             ht is 4*h (w2 was x4): hsub<52 -> bf16, hsub>=52 -> fp8e4
    phase B: po[e,:] = sum_h w3^T @ ht = 512*out    (52 bf16 MMs +
             6 fp8 DoubleRow MMs contracting 2 subtiles each)
             out = po/512 via ACT copy(scale), DMA to DRAM.

The fp8 path puts 12/64 of the down-projection contraction on the
DoubleRow (2x) tensor-engine mode; measured end-to-end l2 error vs the
fp32 reference is ~1.7e-2 (gate: 2e-2).  Scales keep every fp8 value
well under the TRN e4m3 max-normal 240 (conversion overflows to Inf,
not saturation): |4h| <= ~120, |128*w3| <= ~11.
"""

import json

import numpy as np
import ml_dtypes

import concourse.bass as bass
import concourse.mybir as mybir
import concourse.tile as tile
from concourse.vector_clock import ScopedClock
from concourse.bass_utils import run_bass_kernel_spmd

# ---------------------------------------------------------------- shapes
N_CORES = 8
EMB = 2048          # E
HID = 8192          # H
T_TOTAL = 8192      # B*S tokens
T_SHARD = T_TOTAL // N_CORES   # 1024 tokens per core
T_CHUNK = 512                  # tokens per on-chip pass
E_SUB = EMB // 128             # 16 contraction subtiles for phase A
H_SUB = HID // 128             # 64 contraction subtiles for phase B

N_FP8_PAIRS = 6                # DoubleRow pairs in phase B (0 disables)
N_FP8_SUB = 2 * N_FP8_PAIRS    # h-subtiles carried in fp8
N_BF_SUB = H_SUB - N_FP8_SUB   # h-subtiles carried in bf16
W2_SCALE = 4.0                 # ht = 4*h  (max ~120 < 240 fp8 normal max)
W3_SCALE = 128.0               # psum3 = 512*out
OUT_DESCALE = 1.0 / (W2_SCALE * W3_SCALE)

CDT = mybir.dt.bfloat16        # compute dtype on the PE
F8 = mybir.dt.float8e4
NP_CDT = ml_dtypes.bfloat16
NP_F8 = ml_dtypes.float8_e4m3

P = 128
F32 = mybir.dt.float32


class _TileContextSplitWait(tile.TileContext):
    """The walrus build in this environment rejects >1 sync-wait on a
    CTRL (Drain) instruction.  Split the kernel-tail drain's waits into
    single-wait nops emitted just before it."""

    def _drain_and_barrier(self, tick_clock, wait_clock):
        probe = self.nc.sync.nop(nofuse=True)
        wait_clock.add_sem_waits(
            probe.ins, ScopedClock({None: tick_clock.global_clock})
        )
        si = probe.ins.sync_info
        if si is not None and len(si.on_wait) > 1:
            waits = list(si.on_wait)
            probe.ins.sync_info = mybir.SyncInfo(
                on_wait=waits[:1], on_update=list(si.on_update)
            )
            for w in waits[1:]:
                n = self.nc.sync.nop(nofuse=True)
                n.ins.sync_info = mybir.SyncInfo(on_wait=[w], on_update=[])
        self.nc.sync.drain()
        self.nc.all_engine_barrier()
        assert self.sems is not None
        popped = self.nc._tile_sem_poison_stack.pop()
        assert popped is self._sem_poison
        self.nc.clear_and_free_semaphores(list(self.sems.allocated().values()))
        self.nc.all_engine_barrier()


def _split_multi_waits(bir_bytes):
    """The walrus build here accepts at most one sync-wait command per
    instruction (setupSyncWait raises 'Too many sync wait commands').
    Tile attaches however many the dependence analysis needs, so move
    extra waits onto NoOp instructions inserted just before, on the same
    engine's stream -- semantically identical, codegen-compatible."""
    bir = json.loads(bir_bytes)
    for fn in bir["functions"]:
        for blk in fn["blocks"]:
            insts = blk.get("instructions")
            if not insts:
                continue
            out = []
            changed = False
            for inst in insts:
                si = inst.get("sync_info")
                waits = (si or {}).get("on_wait") or []
                if len(waits) > 1:
                    changed = True
                    for j, w in enumerate(waits[:-1]):
                        out.append(
                            {
                                "debug": inst.get("debug"),
                                "engine": inst["engine"],
                                "ins": [],
                                "name": f"{inst['name']}-w{j}",
                                "opcode": "NoOp",
                                "outs": [],
                                "sync_info": {"on_update": [], "on_wait": [w]},
                            }
                        )
                    si["on_wait"] = waits[-1:]
                out.append(inst)
            if changed:
                blk["instructions"] = out
    return json.dumps(bir).encode()


def _build_nc():
    nc = bass.Bass(target_bir_lowering=False)

    xd = nc.dram_tensor(
        "xd", [T_SHARD // T_CHUNK, P, E_SUB, T_CHUNK], CDT, kind="ExternalInput"
    )
    w1d = nc.dram_tensor("w1d", [H_SUB, P, E_SUB, 128], CDT, kind="ExternalInput")
    w2d = nc.dram_tensor("w2d", [H_SUB, P, E_SUB, 128], CDT, kind="ExternalInput")
    w3d = nc.dram_tensor(
        "w3d", [E_SUB, 2, P, N_BF_SUB // 2, 128], CDT, kind="ExternalInput"
    )
    if N_FP8_PAIRS:
        w38d = nc.dram_tensor(
            "w38d", [E_SUB, P, N_FP8_PAIRS, 2, 128], F8, kind="ExternalInput"
        )
    outt = nc.dram_tensor("outt", [EMB, T_SHARD], F32, kind="ExternalOutput")

    n_chunks = T_SHARD // T_CHUNK
    hh = N_BF_SUB // 2  # bf16 subtiles per w3 half-slab

    with _TileContextSplitWait(nc) as tc:
        with (
            tc.tile_pool(name="xp", bufs=1) as xp,
            tc.tile_pool(name="wp", bufs=3) as wp,
            tc.tile_pool(name="w3p", bufs=2) as w3p,
            tc.tile_pool(name="w38p", bufs=3) as w38p,
            tc.tile_pool(name="htp", bufs=1) as htp,
            tc.tile_pool(name="ht8p", bufs=1) as ht8p,
            tc.tile_pool(name="slp", bufs=3) as slp,
            tc.tile_pool(name="op", bufs=4) as op,
            tc.tile_pool(name="ps", bufs=2, space="PSUM") as ps,
        ):
            for c in range(n_chunks):
                t0 = c * T_CHUNK
                # ---- x for this chunk, in 4 parts so the first matmul
                # only waits on 512 KB.
                xparts = [
                    xp.tile([P, 4, T_CHUNK], CDT, name=f"xs{q}") for q in range(4)
                ]
                w1s_first = None
                if c == 0:
                    # startup order: x part0, first w1 slab, x part1,
                    # first w2 slab, x parts 2-3.
                    nc.sync.dma_start(xparts[0][:], xd[0, :, 0:4, :])
                    w1s_first = wp.tile([P, E_SUB, 128], CDT, name="w1s")
                    nc.sync.dma_start(w1s_first[:], w1d[0])
                    nc.sync.dma_start(xparts[1][:], xd[0, :, 4:8, :])
                    w2s_first = wp.tile([P, E_SUB, 128], CDT, name="w2s")
                    nc.sync.dma_start(w2s_first[:], w2d[0])
                    nc.sync.dma_start(xparts[2][:], xd[0, :, 8:12, :])
                    nc.sync.dma_start(xparts[3][:], xd[0, :, 12:16, :])
                else:
                    for q in range(4):
                        nc.sync.dma_start(
                            xparts[q][:], xd[c, :, 4 * q : 4 * q + 4, :]
                        )

                def xview(e):
                    return xparts[e // 4][:, e % 4, :]

                ht = htp.tile([P, N_BF_SUB, T_CHUNK], CDT, name="ht")
                ht8 = [
                    ht8p.tile([P, 2, T_CHUNK], F8, name=f"ht8_{pi}")
                    for pi in range(N_FP8_PAIRS)
                ]

                # ---------------- phase A: gate/up + silu*up -> ht
                for hti in range(H_SUB):
                    if c == 0 and hti == 0:
                        w1s, w2s = w1s_first, w2s_first
                    else:
                        w1s = wp.tile([P, E_SUB, 128], CDT, name="w1s")
                        nc.sync.dma_start(w1s[:], w1d[hti])
                        w2s = wp.tile([P, E_SUB, 128], CDT, name="w2s")
                        nc.sync.dma_start(w2s[:], w2d[hti])

                    pg = ps.tile([P, T_CHUNK], F32, name="pg")
                    for e in range(E_SUB):
                        nc.tensor.matmul(
                            pg[:],
                            w1s[:, e, :],
                            xview(e),
                            start=(e == 0),
                            stop=(e == E_SUB - 1),
                        )
                    pu = ps.tile([P, T_CHUNK], F32, name="pu")
                    for e in range(E_SUB):
                        nc.tensor.matmul(
                            pu[:],
                            w2s[:, e, :],
                            xview(e),
                            start=(e == 0),
                            stop=(e == E_SUB - 1),
                        )
                    sl = slp.tile([P, T_CHUNK], CDT, name="sl")
                    nc.scalar.activation(
                        sl[:], pg[:], mybir.ActivationFunctionType.Silu
                    )
                    if hti < N_BF_SUB:
                        nc.vector.tensor_mul(ht[:, hti, :], sl[:], pu[:])
                    else:
                        pi, j = divmod(hti - N_BF_SUB, 2)
                        nc.vector.tensor_mul(ht8[pi][:, j, :], sl[:], pu[:])

                # ---------------- phase B: po = sum_h w3^T @ ht = 512*out
                for et in range(E_SUB):
                    e0 = et * P
                    w3a = w3p.tile([P, hh, P], CDT, name="w3a")
                    nc.sync.dma_start(w3a[:], w3d[et, 0])
                    w3b = w3p.tile([P, hh, P], CDT, name="w3b")
                    nc.sync.dma_start(w3b[:], w3d[et, 1])
                    if N_FP8_PAIRS:
                        w38t = w38p.tile([P, N_FP8_PAIRS, 2, P], F8, name="w38t")
                        nc.sync.dma_start(w38t[:], w38d[et])
                    po = ps.tile([P, T_CHUNK], F32, name="po")
                    for h in range(N_BF_SUB):
                        w3v = w3a[:, h, :] if h < hh else w3b[:, h - hh, :]
                        nc.tensor.matmul(
                            po[:],
                            w3v,
                            ht[:, h, :],
                            start=(h == 0),
                            stop=(h == N_BF_SUB - 1 and N_FP8_PAIRS == 0),
                        )
                    for pi in range(N_FP8_PAIRS):
                        nc.tensor.matmul(
                            po[:],
                            w38t[:, pi, :, :],
                            ht8[pi][:],
                            start=False,
                            stop=(pi == N_FP8_PAIRS - 1),
                            perf_mode=mybir.MatmulPerfMode.DoubleRow,
                        )
                    # out = po/512, PSUM -> SBUF -> DRAM.  Final tile of
                    # the kernel is split so the tail DMA is small.
                    n_sl = 4 if (c == n_chunks - 1 and et == E_SUB - 1) else 1
                    tsl = T_CHUNK // n_sl
                    for s in range(n_sl):
                        ot = op.tile([P, tsl], F32, name="ot")
                        nc.scalar.activation(
                            ot[:],
                            po[:, s * tsl : (s + 1) * tsl],
                            mybir.ActivationFunctionType.Copy,
                            scale=OUT_DESCALE,
                        )
                        nc.sync.dma_start(
                            outt[
                                e0 : e0 + P,
                                t0 + s * tsl : t0 + (s + 1) * tsl,
                            ],
                            ot[:],
                        )

    fixed = _split_multi_waits(bass.Bass.to_json_bytes(nc))
    nc.to_json_bytes = lambda: fixed
    return nc


_nc_cache = None


def _get_nc():
    global _nc_cache
    if _nc_cache is None:
        _nc_cache = _build_nc()
    return _nc_cache


def _prep_inputs(x, w1, w2, w3):
    n_chunks = T_SHARD // T_CHUNK
    # x: [B,S,E] -> xt [E, T] -> per-core [2, 128, 16, 512]
    xt = x.reshape(T_TOTAL, EMB).T.astype(NP_CDT)  # [E, T_total]

    # w1/w2: [H, E] -> [64 hti, 128 p, 16 s, 128 h] from w.T [E, H]
    def wslab(w):
        a = w.T.reshape(E_SUB, P, H_SUB, P)  # [s, p, hti, h]
        return np.ascontiguousarray(a.transpose(2, 1, 0, 3))

    w1d = wslab(w1.astype(NP_CDT))
    w2d = wslab((w2 * W2_SCALE).astype(NP_CDT))

    # w3: [E, H] -> w3T [H, E] scaled x128
    w3t = (w3.T * W3_SCALE).astype(np.float32)  # [H, E]
    a = w3t.reshape(H_SUB, P, E_SUB, P)  # [s, p, et, e]
    bf = a[:N_BF_SUB].astype(NP_CDT)  # [52, 128, 16, 128]
    w3d = np.ascontiguousarray(
        bf.reshape(2, N_BF_SUB // 2, P, E_SUB, P).transpose(3, 0, 2, 1, 4)
    )  # [et, half, p, s, e]
    w38d = None
    if N_FP8_PAIRS:
        f8 = a[N_BF_SUB:].astype(NP_F8)  # [12, 128, 16, 128]
        w38d = np.ascontiguousarray(
            f8.reshape(N_FP8_PAIRS, 2, P, E_SUB, P).transpose(3, 2, 0, 1, 4)
        )  # [et, p, pi, j, e]

    in_maps = []
    for i in range(N_CORES):
        sh = xt[:, i * T_SHARD : (i + 1) * T_SHARD]  # [E, 1024]
        xdc = np.ascontiguousarray(
            sh.reshape(E_SUB, P, n_chunks, T_CHUNK).transpose(2, 1, 0, 3)
        )  # [c, p, s, t]
        m = {"xd": xdc, "w1d": w1d, "w2d": w2d, "w3d": w3d}
        if N_FP8_PAIRS:
            m["w38d"] = w38d
        in_maps.append(m)
    return in_maps


def kernel(x, w1, w2, w3, scale_x=None, _trace=False):
    x = np.asarray(x, np.float32)
    w1 = np.asarray(w1, np.float32)
    w2 = np.asarray(w2, np.float32)
    w3 = np.asarray(w3, np.float32)

    nc = _get_nc()
    in_maps = _prep_inputs(x, w1, w2, w3)
    res = run_bass_kernel_spmd(nc, in_maps, list(range(N_CORES)), trace=_trace)

    outt = np.concatenate(
        [np.asarray(res.results[i]["outt"]) for i in range(N_CORES)], axis=1
    )  # [E, T_total]
    out = np.ascontiguousarray(outt.T).reshape(4, 2048, EMB).astype(np.float32)
    if _trace:
        kernel.last_results = res
    return out


if __name__ == "__main__":
    rng = np.random.default_rng(0)
    x = rng.standard_normal((4, 2048, EMB), dtype=np.float32)
    w1 = (rng.standard_normal((HID, EMB), dtype=np.float32) * 0.03).astype(
        np.float32
    )
    w2 = (rng.standard_normal((HID, EMB), dtype=np.float32) * 0.03).astype(
        np.float32
    )
    w3 = (rng.standard_normal((EMB, HID), dtype=np.float32) * 0.015).astype(
        np.float32
    )
    out = kernel(x, w1, w2, w3)
    print("out", out.shape, out.dtype, float(np.abs(out).mean()))


# revision 7
# speedup vs baseline: 1.0390x; 1.0055x over previous
"""Trainium2 Bass kernel for a SwiGLU-style feed-forward block.

reference:
    gate = x @ w1.T ; up = x @ w2.T ; h = silu(gate) * up ; out = h @ w3.T
    x: [4, 2048, 2048] f32, w1/w2: [8192, 2048] f32, w3: [2048, 8192] f32

Strategy: pure data-parallel over the 8192 tokens -- each of the 8
NeuronCores gets 1024 tokens and the full weights, processed in two
512-token chunks.  All tensors are pre-transposed on the host into
partition-major slabs so every DMA is 4-16 KB contiguous per partition:

    xd   [2, 128, 16, 512]      activations, feature-major, per chunk
    w1d  [64, 128, 16, 128]     one slab per 128-wide h-tile
    w2d  [64, 128, 16, 128]     scaled x4 (lossless in bf16)
    w3d  [16, 2, 128, 26, 128]  bf16 down-proj, hsub 0..51, scaled x128
    w38d [16, 128, 6, 2, 128]   fp8e4 down-proj pairs, hsub 52..63, x128

Per chunk:
    phase A: gateT/upT[h,:] = w1/w2-slab^T @ x      (PSUM, fp32)
             sl = silu(gate) [ACT];  ht = sl * pu   [DVE]
             ht is 4*h (w2 was x4): hsub<52 -> bf16, hsub>=52 -> fp8e4
    phase B: po[e,:] = sum_h w3^T @ ht = 512*out    (52 bf16 MMs +
             6 fp8 DoubleRow MMs contracting 2 subtiles each)
             out = po/512 via ACT copy(scale), DMA to DRAM.

The fp8 path puts 12/64 of the down-projection contraction on the
DoubleRow (2x) tensor-engine mode; measured end-to-end l2 error vs the
fp32 reference is ~1.7e-2 (gate: 2e-2).  Scales keep every fp8 value
well under the TRN e4m3 max-normal 240 (conversion overflows to Inf,
not saturation): |4h| <= ~120, |128*w3| <= ~11.
"""

import json

import numpy as np
import ml_dtypes

import concourse.bass as bass
import concourse.mybir as mybir
import concourse.tile as tile
from concourse.vector_clock import ScopedClock
from concourse.bass_utils import run_bass_kernel_spmd

# ---------------------------------------------------------------- shapes
N_CORES = 8
EMB = 2048          # E
HID = 8192          # H
T_TOTAL = 8192      # B*S tokens
T_SHARD = T_TOTAL // N_CORES   # 1024 tokens per core
T_CHUNK = 512                  # tokens per on-chip pass
E_SUB = EMB // 128             # 16 contraction subtiles for phase A
H_SUB = HID // 128             # 64 contraction subtiles for phase B

N_FP8_PAIRS = 7                # DoubleRow pairs in phase B (0 disables)
N_FP8_SUB = 2 * N_FP8_PAIRS    # h-subtiles carried in fp8
N_BF_SUB = H_SUB - N_FP8_SUB   # h-subtiles carried in bf16
W2_SCALE = 4.0                 # ht = 4*h  (max ~120 < 240 fp8 normal max)
W3_SCALE = 128.0               # psum3 = 512*out
OUT_DESCALE = 1.0 / (W2_SCALE * W3_SCALE)

CDT = mybir.dt.bfloat16        # compute dtype on the PE
F8 = mybir.dt.float8e4
NP_CDT = ml_dtypes.bfloat16
NP_F8 = ml_dtypes.float8_e4m3

P = 128
F32 = mybir.dt.float32


class _TileContextSplitWait(tile.TileContext):
    """The walrus build in this environment rejects >1 sync-wait on a
    CTRL (Drain) instruction.  Split the kernel-tail drain's waits into
    single-wait nops emitted just before it."""

    def _drain_and_barrier(self, tick_clock, wait_clock):
        probe = self.nc.sync.nop(nofuse=True)
        wait_clock.add_sem_waits(
            probe.ins, ScopedClock({None: tick_clock.global_clock})
        )
        si = probe.ins.sync_info
        if si is not None and len(si.on_wait) > 1:
            waits = list(si.on_wait)
            probe.ins.sync_info = mybir.SyncInfo(
                on_wait=waits[:1], on_update=list(si.on_update)
            )
            for w in waits[1:]:
                n = self.nc.sync.nop(nofuse=True)
                n.ins.sync_info = mybir.SyncInfo(on_wait=[w], on_update=[])
        self.nc.sync.drain()
        self.nc.all_engine_barrier()
        assert self.sems is not None
        popped = self.nc._tile_sem_poison_stack.pop()
        assert popped is self._sem_poison
        self.nc.clear_and_free_semaphores(list(self.sems.allocated().values()))
        self.nc.all_engine_barrier()


def _split_multi_waits(bir_bytes):
    """The walrus build here accepts at most one sync-wait command per
    instruction (setupSyncWait raises 'Too many sync wait commands').
    Tile attaches however many the dependence analysis needs, so move
    extra waits onto NoOp instructions inserted just before, on the same
    engine's stream -- semantically identical, codegen-compatible."""
    bir = json.loads(bir_bytes)
    for fn in bir["functions"]:
        for blk in fn["blocks"]:
            insts = blk.get("instructions")
            if not insts:
                continue
            out = []
            changed = False
            for inst in insts:
                si = inst.get("sync_info")
                waits = (si or {}).get("on_wait") or []
                if len(waits) > 1:
                    changed = True
                    for j, w in enumerate(waits[:-1]):
                        out.append(
                            {
                                "debug": inst.get("debug"),
                                "engine": inst["engine"],
                                "ins": [],
                                "name": f"{inst['name']}-w{j}",
                                "opcode": "NoOp",
                                "outs": [],
                                "sync_info": {"on_update": [], "on_wait": [w]},
                            }
                        )
                    si["on_wait"] = waits[-1:]
                out.append(inst)
            if changed:
                blk["instructions"] = out
    return json.dumps(bir).encode()


def _build_nc():
    nc = bass.Bass(target_bir_lowering=False)

    xd = nc.dram_tensor(
        "xd", [T_SHARD // T_CHUNK, P, E_SUB, T_CHUNK], CDT, kind="ExternalInput"
    )
    w1d = nc.dram_tensor("w1d", [H_SUB, P, E_SUB, 128], CDT, kind="ExternalInput")
    w2d = nc.dram_tensor("w2d", [H_SUB, P, E_SUB, 128], CDT, kind="ExternalInput")
    w3d = nc.dram_tensor(
        "w3d", [E_SUB, 2, P, N_BF_SUB // 2, 128], CDT, kind="ExternalInput"
    )
    if N_FP8_PAIRS:
        w38d = nc.dram_tensor(
            "w38d", [E_SUB, P, N_FP8_PAIRS, 2, 128], F8, kind="ExternalInput"
        )
    outt = nc.dram_tensor("outt", [EMB, T_SHARD], F32, kind="ExternalOutput")

    n_chunks = T_SHARD // T_CHUNK
    hh = N_BF_SUB // 2  # bf16 subtiles per w3 half-slab

    with _TileContextSplitWait(nc) as tc:
        with (
            tc.tile_pool(name="xp", bufs=1) as xp,
            tc.tile_pool(name="wp", bufs=3) as wp,
            tc.tile_pool(name="w3p", bufs=2) as w3p,
            tc.tile_pool(name="w38p", bufs=3) as w38p,
            tc.tile_pool(name="htp", bufs=1) as htp,
            tc.tile_pool(name="ht8p", bufs=1) as ht8p,
            tc.tile_pool(name="slp", bufs=3) as slp,
            tc.tile_pool(name="op", bufs=4) as op,
            tc.tile_pool(name="ps", bufs=2, space="PSUM") as ps,
        ):
            for c in range(n_chunks):
                t0 = c * T_CHUNK
                w1s_first = None
                if c == 0:
                    # Startup: fine-grained leading pieces so the first
                    # matmul only waits on ~384 KB, then overlap the
                    # rest of x/w1/w2 with the cold-clock matmuls.
                    xsplit = (2, 6, 8)          # e-subtiles per piece
                    wsplit = (4, 12)
                    xparts, xmap = [], []
                    e0_ = 0
                    for q, ne in enumerate(xsplit):
                        xt_ = xp.tile([P, ne, T_CHUNK], CDT, name=f"xs{q}")
                        xparts.append(xt_)
                        xmap += [(q, e0_)] * ne
                        e0_ += ne
                    w1a = wp.tile([P, wsplit[0], 128], CDT, name="w1sa")
                    w1b = wp.tile([P, wsplit[1], 128], CDT, name="w1sb")
                    nc.sync.dma_start(xparts[0][:], xd[0, :, 0:2, :])
                    nc.sync.dma_start(w1a[:], w1d[0, :, 0 : wsplit[0], :])
                    nc.sync.dma_start(xparts[1][:], xd[0, :, 2:8, :])
                    nc.sync.dma_start(w1b[:], w1d[0, :, wsplit[0] :, :])
                    w2s_first = wp.tile([P, E_SUB, 128], CDT, name="w2s")
                    nc.sync.dma_start(w2s_first[:], w2d[0])
                    nc.sync.dma_start(xparts[2][:], xd[0, :, 8:16, :])

                    def w1view_first(e):
                        if e < wsplit[0]:
                            return w1a[:, e, :]
                        return w1b[:, e - wsplit[0], :]

                    w1s_first = w1view_first
                else:
                    xparts, xmap = [], []
                    for q in range(4):
                        xt_ = xp.tile([P, 4, T_CHUNK], CDT, name=f"xs{q}")
                        nc.sync.dma_start(xt_[:], xd[c, :, 4 * q : 4 * q + 4, :])
                        xparts.append(xt_)
                        xmap += [(q, 4 * q)] * 4

                def xview(e):
                    q, e0_ = xmap[e]
                    return xparts[q][:, e - e0_, :]

                ht = htp.tile([P, N_BF_SUB, T_CHUNK], CDT, name="ht")
                ht8 = [
                    ht8p.tile([P, 2, T_CHUNK], F8, name=f"ht8_{pi}")
                    for pi in range(N_FP8_PAIRS)
                ]

                # ---------------- phase A: gate/up + silu*up -> ht
                for hti in range(H_SUB):
                    if c == 0 and hti == 0:
                        w1v, w2s = w1s_first, w2s_first
                    else:
                        w1s = wp.tile([P, E_SUB, 128], CDT, name="w1s")
                        nc.sync.dma_start(w1s[:], w1d[hti])
                        w2s = wp.tile([P, E_SUB, 128], CDT, name="w2s")
                        nc.sync.dma_start(w2s[:], w2d[hti])

                        def w1v(e, w1s=w1s):
                            return w1s[:, e, :]

                    pg = ps.tile([P, T_CHUNK], F32, name="pg")
                    for e in range(E_SUB):
                        nc.tensor.matmul(
                            pg[:],
                            w1v(e),
                            xview(e),
                            start=(e == 0),
                            stop=(e == E_SUB - 1),
                        )
                    pu = ps.tile([P, T_CHUNK], F32, name="pu")
                    for e in range(E_SUB):
                        nc.tensor.matmul(
                            pu[:],
                            w2s[:, e, :],
                            xview(e),
                            start=(e == 0),
                            stop=(e == E_SUB - 1),
                        )
                    sl = slp.tile([P, T_CHUNK], CDT, name="sl")
                    nc.scalar.activation(
                        sl[:], pg[:], mybir.ActivationFunctionType.Silu
                    )
                    if hti < N_BF_SUB:
                        nc.vector.tensor_mul(ht[:, hti, :], sl[:], pu[:])
                    else:
                        pi, j = divmod(hti - N_BF_SUB, 2)
                        nc.vector.tensor_mul(ht8[pi][:, j, :], sl[:], pu[:])

                # ---------------- phase B: po = sum_h w3^T @ ht = 512*out
                for et in range(E_SUB):
                    e0 = et * P
                    w3a = w3p.tile([P, hh, P], CDT, name="w3a")
                    nc.sync.dma_start(w3a[:], w3d[et, 0])
                    w3b = w3p.tile([P, hh, P], CDT, name="w3b")
                    nc.sync.dma_start(w3b[:], w3d[et, 1])
                    if N_FP8_PAIRS:
                        w38t = w38p.tile([P, N_FP8_PAIRS, 2, P], F8, name="w38t")
                        nc.sync.dma_start(w38t[:], w38d[et])
                    po = ps.tile([P, T_CHUNK], F32, name="po")
                    for h in range(N_BF_SUB):
                        w3v = w3a[:, h, :] if h < hh else w3b[:, h - hh, :]
                        nc.tensor.matmul(
                            po[:],
                            w3v,
                            ht[:, h, :],
                            start=(h == 0),
                            stop=(h == N_BF_SUB - 1 and N_FP8_PAIRS == 0),
                        )
                    for pi in range(N_FP8_PAIRS):
                        nc.tensor.matmul(
                            po[:],
                            w38t[:, pi, :, :],
                            ht8[pi][:],
                            start=False,
                            stop=(pi == N_FP8_PAIRS - 1),
                            perf_mode=mybir.MatmulPerfMode.DoubleRow,
                        )
                    # out = po/512, PSUM -> SBUF -> DRAM.  Final tile of
                    # the kernel is split so the tail DMA is small.
                    n_sl = 2 if (c == n_chunks - 1 and et == E_SUB - 1) else 1
                    tsl = T_CHUNK // n_sl
                    for s in range(n_sl):
                        ot = op.tile([P, tsl], F32, name="ot")
                        nc.scalar.activation(
                            ot[:],
                            po[:, s * tsl : (s + 1) * tsl],
                            mybir.ActivationFunctionType.Copy,
                            scale=OUT_DESCALE,
                        )
                        nc.sync.dma_start(
                            outt[
                                e0 : e0 + P,
                                t0 + s * tsl : t0 + (s + 1) * tsl,
                            ],
                            ot[:],
                        )

    fixed = _split_multi_waits(bass.Bass.to_json_bytes(nc))
    nc.to_json_bytes = lambda: fixed
    return nc


_nc_cache = None


def _get_nc():
    global _nc_cache
    if _nc_cache is None:
        _nc_cache = _build_nc()
    return _nc_cache


def _prep_inputs(x, w1, w2, w3):
    n_chunks = T_SHARD // T_CHUNK
    # x: [B,S,E] -> xt [E, T] -> per-core [2, 128, 16, 512]
    xt = x.reshape(T_TOTAL, EMB).T.astype(NP_CDT)  # [E, T_total]

    # w1/w2: [H, E] -> [64 hti, 128 p, 16 s, 128 h] from w.T [E, H]
    def wslab(w):
        a = w.T.reshape(E_SUB, P, H_SUB, P)  # [s, p, hti, h]
        return np.ascontiguousarray(a.transpose(2, 1, 0, 3))

    w1d = wslab(w1.astype(NP_CDT))
    w2d = wslab((w2 * W2_SCALE).astype(NP_CDT))

    # w3: [E, H] -> w3T [H, E] scaled x128
    w3t = (w3.T * W3_SCALE).astype(np.float32)  # [H, E]
    a = w3t.reshape(H_SUB, P, E_SUB, P)  # [s, p, et, e]
    bf = a[:N_BF_SUB].astype(NP_CDT)  # [52, 128, 16, 128]
    w3d = np.ascontiguousarray(
        bf.reshape(2, N_BF_SUB // 2, P, E_SUB, P).transpose(3, 0, 2, 1, 4)
    )  # [et, half, p, s, e]
    w38d = None
    if N_FP8_PAIRS:
        f8 = a[N_BF_SUB:].astype(NP_F8)  # [12, 128, 16, 128]
        w38d = np.ascontiguousarray(
            f8.reshape(N_FP8_PAIRS, 2, P, E_SUB, P).transpose(3, 2, 0, 1, 4)
        )  # [et, p, pi, j, e]

    in_maps = []
    for i in range(N_CORES):
        sh = xt[:, i * T_SHARD : (i + 1) * T_SHARD]  # [E, 1024]
        xdc = np.ascontiguousarray(
            sh.reshape(E_SUB, P, n_chunks, T_CHUNK).transpose(2, 1, 0, 3)
        )  # [c, p, s, t]
        m = {"xd": xdc, "w1d": w1d, "w2d": w2d, "w3d": w3d}
        if N_FP8_PAIRS:
            m["w38d"] = w38d
        in_maps.append(m)
    return in_maps


def kernel(x, w1, w2, w3, scale_x=None, _trace=False):
    x = np.asarray(x, np.float32)
    w1 = np.asarray(w1, np.float32)
    w2 = np.asarray(w2, np.float32)
    w3 = np.asarray(w3, np.float32)

    nc = _get_nc()
    in_maps = _prep_inputs(x, w1, w2, w3)
    res = run_bass_kernel_spmd(nc, in_maps, list(range(N_CORES)), trace=_trace)

    outt = np.concatenate(
        [np.asarray(res.results[i]["outt"]) for i in range(N_CORES)], axis=1
    )  # [E, T_total]
    out = np.ascontiguousarray(outt.T).reshape(4, 2048, EMB).astype(np.float32)
    if _trace:
        kernel.last_results = res
    return out


if __name__ == "__main__":
    rng = np.random.default_rng(0)
    x = rng.standard_normal((4, 2048, EMB), dtype=np.float32)
    w1 = (rng.standard_normal((HID, EMB), dtype=np.float32) * 0.03).astype(
        np.float32
    )
    w2 = (rng.standard_normal((HID, EMB), dtype=np.float32) * 0.03).astype(
        np.float32
    )
    w3 = (rng.standard_normal((EMB, HID), dtype=np.float32) * 0.015).astype(
        np.float32
    )
    out = kernel(x, w1, w2, w3)
    print("out", out.shape, out.dtype, float(np.abs(out).mean()))


# revision 8
# speedup vs baseline: 1.0442x; 1.0050x over previous
"""Trainium2 Bass kernel for a SwiGLU-style feed-forward block.

reference:
    gate = x @ w1.T ; up = x @ w2.T ; h = silu(gate) * up ; out = h @ w3.T
    x: [4, 2048, 2048] f32, w1/w2: [8192, 2048] f32, w3: [2048, 8192] f32

Strategy: pure data-parallel over the 8192 tokens -- each of the 8
NeuronCores gets 1024 tokens and the full weights, processed in two
512-token chunks.  All tensors are pre-transposed on the host into
partition-major slabs so every DMA is 4-16 KB contiguous per partition:

    xd   [2, 128, 16, 512]      activations, feature-major, per chunk
    w1d  [64, 128, 16, 128]     one slab per 128-wide h-tile
    w2d  [64, 128, 16, 128]     scaled x4 (lossless in bf16)
    w3d  [16, 2, 128, 26, 128]  bf16 down-proj, hsub 0..51, scaled x128
    w38d [16, 128, 6, 2, 128]   fp8e4 down-proj pairs, hsub 52..63, x128

Per chunk:
    phase A: gateT/upT[h,:] = w1/w2-slab^T @ x      (PSUM, fp32)
             sl = silu(gate) [ACT];  ht = sl * pu   [DVE]
             ht is 4*h (w2 was x4): hsub<52 -> bf16, hsub>=52 -> fp8e4
    phase B: po[e,:] = sum_h w3^T @ ht = 512*out    (52 bf16 MMs +
             6 fp8 DoubleRow MMs contracting 2 subtiles each)
             out = po/512 via ACT copy(scale), DMA to DRAM.

The fp8 path puts 12/64 of the down-projection contraction on the
DoubleRow (2x) tensor-engine mode; measured end-to-end l2 error vs the
fp32 reference is ~1.7e-2 (gate: 2e-2).  Scales keep every fp8 value
well under the TRN e4m3 max-normal 240 (conversion overflows to Inf,
not saturation): |4h| <= ~120, |128*w3| <= ~11.
"""

import json

import numpy as np
import ml_dtypes

import concourse.bass as bass
import concourse.mybir as mybir
import concourse.tile as tile
from concourse.vector_clock import ScopedClock
from concourse.bass_utils import run_bass_kernel_spmd

# ---------------------------------------------------------------- shapes
N_CORES = 8
EMB = 2048          # E
HID = 8192          # H
T_TOTAL = 8192      # B*S tokens
T_SHARD = T_TOTAL // N_CORES   # 1024 tokens per core
T_CHUNK = 512                  # tokens per on-chip pass
E_SUB = EMB // 128             # 16 contraction subtiles for phase A
H_SUB = HID // 128             # 64 contraction subtiles for phase B

N_FP8_PAIRS = 8                # DoubleRow pairs in phase B (0 disables)
N_FP8_SUB = 2 * N_FP8_PAIRS    # h-subtiles carried in fp8
N_BF_SUB = H_SUB - N_FP8_SUB   # h-subtiles carried in bf16
W2_SCALE = 4.0                 # ht = 4*h  (max ~120 < 240 fp8 normal max)
W3_SCALE = 128.0               # psum3 = 512*out
OUT_DESCALE = 1.0 / (W2_SCALE * W3_SCALE)

CDT = mybir.dt.bfloat16        # compute dtype on the PE
F8 = mybir.dt.float8e4
NP_CDT = ml_dtypes.bfloat16
NP_F8 = ml_dtypes.float8_e4m3

P = 128
F32 = mybir.dt.float32


class _TileContextSplitWait(tile.TileContext):
    """The walrus build in this environment rejects >1 sync-wait on a
    CTRL (Drain) instruction.  Split the kernel-tail drain's waits into
    single-wait nops emitted just before it."""

    def _drain_and_barrier(self, tick_clock, wait_clock):
        probe = self.nc.sync.nop(nofuse=True)
        wait_clock.add_sem_waits(
            probe.ins, ScopedClock({None: tick_clock.global_clock})
        )
        si = probe.ins.sync_info
        if si is not None and len(si.on_wait) > 1:
            waits = list(si.on_wait)
            probe.ins.sync_info = mybir.SyncInfo(
                on_wait=waits[:1], on_update=list(si.on_update)
            )
            for w in waits[1:]:
                n = self.nc.sync.nop(nofuse=True)
                n.ins.sync_info = mybir.SyncInfo(on_wait=[w], on_update=[])
        self.nc.sync.drain()
        self.nc.all_engine_barrier()
        assert self.sems is not None
        popped = self.nc._tile_sem_poison_stack.pop()
        assert popped is self._sem_poison
        self.nc.clear_and_free_semaphores(list(self.sems.allocated().values()))
        self.nc.all_engine_barrier()


def _split_multi_waits(bir_bytes):
    """The walrus build here accepts at most one sync-wait command per
    instruction (setupSyncWait raises 'Too many sync wait commands').
    Tile attaches however many the dependence analysis needs, so move
    extra waits onto NoOp instructions inserted just before, on the same
    engine's stream -- semantically identical, codegen-compatible."""
    bir = json.loads(bir_bytes)
    for fn in bir["functions"]:
        for blk in fn["blocks"]:
            insts = blk.get("instructions")
            if not insts:
                continue
            out = []
            changed = False
            for inst in insts:
                si = inst.get("sync_info")
                waits = (si or {}).get("on_wait") or []
                if len(waits) > 1:
                    changed = True
                    for j, w in enumerate(waits[:-1]):
                        out.append(
                            {
                                "debug": inst.get("debug"),
                                "engine": inst["engine"],
                                "ins": [],
                                "name": f"{inst['name']}-w{j}",
                                "opcode": "NoOp",
                                "outs": [],
                                "sync_info": {"on_update": [], "on_wait": [w]},
                            }
                        )
                    si["on_wait"] = waits[-1:]
                out.append(inst)
            if changed:
                blk["instructions"] = out
    return json.dumps(bir).encode()


def _build_nc():
    nc = bass.Bass(target_bir_lowering=False)

    xd = nc.dram_tensor(
        "xd", [T_SHARD // T_CHUNK, P, E_SUB, T_CHUNK], CDT, kind="ExternalInput"
    )
    w1d = nc.dram_tensor("w1d", [H_SUB, P, E_SUB, 128], CDT, kind="ExternalInput")
    w2d = nc.dram_tensor("w2d", [H_SUB, P, E_SUB, 128], CDT, kind="ExternalInput")
    w3d = nc.dram_tensor(
        "w3d", [E_SUB, 2, P, N_BF_SUB // 2, 128], CDT, kind="ExternalInput"
    )
    if N_FP8_PAIRS:
        w38d = nc.dram_tensor(
            "w38d", [E_SUB, P, N_FP8_PAIRS, 2, 128], F8, kind="ExternalInput"
        )
    outt = nc.dram_tensor("outt", [EMB, T_SHARD], F32, kind="ExternalOutput")

    n_chunks = T_SHARD // T_CHUNK
    hh = N_BF_SUB // 2  # bf16 subtiles per w3 half-slab

    with _TileContextSplitWait(nc) as tc:
        with (
            tc.tile_pool(name="xp", bufs=1) as xp,
            tc.tile_pool(name="wp", bufs=3) as wp,
            tc.tile_pool(name="w3p", bufs=2) as w3p,
            tc.tile_pool(name="w38p", bufs=3) as w38p,
            tc.tile_pool(name="htp", bufs=1) as htp,
            tc.tile_pool(name="ht8p", bufs=1) as ht8p,
            tc.tile_pool(name="slp", bufs=3) as slp,
            tc.tile_pool(name="op", bufs=4) as op,
            tc.tile_pool(name="ps", bufs=2, space="PSUM") as ps,
        ):
            for c in range(n_chunks):
                t0 = c * T_CHUNK
                w1s_first = None
                if c == 0:
                    # Startup: fine-grained leading pieces so the first
                    # matmul only waits on ~384 KB, then overlap the
                    # rest of x/w1/w2 with the cold-clock matmuls.
                    xsplit = (2, 6, 8)          # e-subtiles per piece
                    wsplit = (4, 12)
                    xparts, xmap = [], []
                    e0_ = 0
                    for q, ne in enumerate(xsplit):
                        xt_ = xp.tile([P, ne, T_CHUNK], CDT, name=f"xs{q}")
                        xparts.append(xt_)
                        xmap += [(q, e0_)] * ne
                        e0_ += ne
                    w1a = wp.tile([P, wsplit[0], 128], CDT, name="w1sa")
                    w1b = wp.tile([P, wsplit[1], 128], CDT, name="w1sb")
                    nc.sync.dma_start(xparts[0][:], xd[0, :, 0:2, :])
                    nc.sync.dma_start(w1a[:], w1d[0, :, 0 : wsplit[0], :])
                    nc.sync.dma_start(xparts[1][:], xd[0, :, 2:8, :])
                    nc.sync.dma_start(w1b[:], w1d[0, :, wsplit[0] :, :])
                    w2s_first = wp.tile([P, E_SUB, 128], CDT, name="w2s")
                    nc.sync.dma_start(w2s_first[:], w2d[0])
                    nc.sync.dma_start(xparts[2][:], xd[0, :, 8:16, :])

                    def w1view_first(e):
                        if e < wsplit[0]:
                            return w1a[:, e, :]
                        return w1b[:, e - wsplit[0], :]

                    w1s_first = w1view_first
                else:
                    xparts, xmap = [], []
                    for q in range(4):
                        xt_ = xp.tile([P, 4, T_CHUNK], CDT, name=f"xs{q}")
                        nc.sync.dma_start(xt_[:], xd[c, :, 4 * q : 4 * q + 4, :])
                        xparts.append(xt_)
                        xmap += [(q, 4 * q)] * 4

                def xview(e):
                    q, e0_ = xmap[e]
                    return xparts[q][:, e - e0_, :]

                ht = htp.tile([P, N_BF_SUB, T_CHUNK], CDT, name="ht")
                ht8 = [
                    ht8p.tile([P, 2, T_CHUNK], F8, name=f"ht8_{pi}")
                    for pi in range(N_FP8_PAIRS)
                ]

                # ---------------- phase A: gate/up + silu*up -> ht
                for hti in range(H_SUB):
                    if c == 0 and hti == 0:
                        w1v, w2s = w1s_first, w2s_first
                    else:
                        w1s = wp.tile([P, E_SUB, 128], CDT, name="w1s")
                        nc.sync.dma_start(w1s[:], w1d[hti])
                        w2s = wp.tile([P, E_SUB, 128], CDT, name="w2s")
                        nc.sync.dma_start(w2s[:], w2d[hti])

                        def w1v(e, w1s=w1s):
                            return w1s[:, e, :]

                    pg = ps.tile([P, T_CHUNK], F32, name="pg")
                    for e in range(E_SUB):
                        nc.tensor.matmul(
                            pg[:],
                            w1v(e),
                            xview(e),
                            start=(e == 0),
                            stop=(e == E_SUB - 1),
                        )
                    pu = ps.tile([P, T_CHUNK], F32, name="pu")
                    for e in range(E_SUB):
                        nc.tensor.matmul(
                            pu[:],
                            w2s[:, e, :],
                            xview(e),
                            start=(e == 0),
                            stop=(e == E_SUB - 1),
                        )
                    sl = slp.tile([P, T_CHUNK], CDT, name="sl")
                    nc.scalar.activation(
                        sl[:], pg[:], mybir.ActivationFunctionType.Silu
                    )
                    if hti < N_BF_SUB:
                        nc.vector.tensor_mul(ht[:, hti, :], sl[:], pu[:])
                    else:
                        pi, j = divmod(hti - N_BF_SUB, 2)
                        nc.vector.tensor_mul(ht8[pi][:, j, :], sl[:], pu[:])

                # ---------------- phase B: po = sum_h w3^T @ ht = 512*out
                for et in range(E_SUB):
                    e0 = et * P
                    w3a = w3p.tile([P, hh, P], CDT, name="w3a")
                    nc.sync.dma_start(w3a[:], w3d[et, 0])
                    w3b = w3p.tile([P, hh, P], CDT, name="w3b")
                    nc.sync.dma_start(w3b[:], w3d[et, 1])
                    if N_FP8_PAIRS:
                        w38t = w38p.tile([P, N_FP8_PAIRS, 2, P], F8, name="w38t")
                        nc.sync.dma_start(w38t[:], w38d[et])
                    po = ps.tile([P, T_CHUNK], F32, name="po")
                    for h in range(N_BF_SUB):
                        w3v = w3a[:, h, :] if h < hh else w3b[:, h - hh, :]
                        nc.tensor.matmul(
                            po[:],
                            w3v,
                            ht[:, h, :],
                            start=(h == 0),
                            stop=(h == N_BF_SUB - 1 and N_FP8_PAIRS == 0),
                        )
                    for pi in range(N_FP8_PAIRS):
                        nc.tensor.matmul(
                            po[:],
                            w38t[:, pi, :, :],
                            ht8[pi][:],
                            start=False,
                            stop=(pi == N_FP8_PAIRS - 1),
                            perf_mode=mybir.MatmulPerfMode.DoubleRow,
                        )
                    # out = po/512, PSUM -> SBUF -> DRAM.  Final tile of
                    # the kernel is split so the tail DMA is small.
                    n_sl = 2 if (c == n_chunks - 1 and et == E_SUB - 1) else 1
                    tsl = T_CHUNK // n_sl
                    for s in range(n_sl):
                        ot = op.tile([P, tsl], F32, name="ot")
                        nc.scalar.activation(
                            ot[:],
                            po[:, s * tsl : (s + 1) * tsl],
                            mybir.ActivationFunctionType.Copy,
                            scale=OUT_DESCALE,
                        )
                        nc.sync.dma_start(
                            outt[
                                e0 : e0 + P,
                                t0 + s * tsl : t0 + (s + 1) * tsl,
                            ],
                            ot[:],
                        )

    fixed = _split_multi_waits(bass.Bass.to_json_bytes(nc))
    nc.to_json_bytes = lambda: fixed
    return nc


_nc_cache = None


def _get_nc():
    global _nc_cache
    if _nc_cache is None:
        _nc_cache = _build_nc()
    return _nc_cache


def _prep_inputs(x, w1, w2, w3):
    n_chunks = T_SHARD // T_CHUNK
    # x: [B,S,E] -> xt [E, T] -> per-core [2, 128, 16, 512]
    xt = x.reshape(T_TOTAL, EMB).T.astype(NP_CDT)  # [E, T_total]

    # w1/w2: [H, E] -> [64 hti, 128 p, 16 s, 128 h] from w.T [E, H]
    def wslab(w):
        a = w.T.reshape(E_SUB, P, H_SUB, P)  # [s, p, hti, h]
        return np.ascontiguousarray(a.transpose(2, 1, 0, 3))

    w1d = wslab(w1.astype(NP_CDT))
    w2d = wslab((w2 * W2_SCALE).astype(NP_CDT))

    # w3: [E, H] -> w3T [H, E] scaled x128
    w3t = (w3.T * W3_SCALE).astype(np.float32)  # [H, E]
    a = w3t.reshape(H_SUB, P, E_SUB, P)  # [s, p, et, e]
    bf = a[:N_BF_SUB].astype(NP_CDT)  # [52, 128, 16, 128]
    w3d = np.ascontiguousarray(
        bf.reshape(2, N_BF_SUB // 2, P, E_SUB, P).transpose(3, 0, 2, 1, 4)
    )  # [et, half, p, s, e]
    w38d = None
    if N_FP8_PAIRS:
        f8 = a[N_BF_SUB:].astype(NP_F8)  # [12, 128, 16, 128]
        w38d = np.ascontiguousarray(
            f8.reshape(N_FP8_PAIRS, 2, P, E_SUB, P).transpose(3, 2, 0, 1, 4)
        )  # [et, p, pi, j, e]

    in_maps = []
    for i in range(N_CORES):
        sh = xt[:, i * T_SHARD : (i + 1) * T_SHARD]  # [E, 1024]
        xdc = np.ascontiguousarray(
            sh.reshape(E_SUB, P, n_chunks, T_CHUNK).transpose(2, 1, 0, 3)
        )  # [c, p, s, t]
        m = {"xd": xdc, "w1d": w1d, "w2d": w2d, "w3d": w3d}
        if N_FP8_PAIRS:
            m["w38d"] = w38d
        in_maps.append(m)
    return in_maps


def kernel(x, w1, w2, w3, scale_x=None, _trace=False):
    x = np.asarray(x, np.float32)
    w1 = np.asarray(w1, np.float32)
    w2 = np.asarray(w2, np.float32)
    w3 = np.asarray(w3, np.float32)

    nc = _get_nc()
    in_maps = _prep_inputs(x, w1, w2, w3)
    res = run_bass_kernel_spmd(nc, in_maps, list(range(N_CORES)), trace=_trace)

    outt = np.concatenate(
        [np.asarray(res.results[i]["outt"]) for i in range(N_CORES)], axis=1
    )  # [E, T_total]
    out = np.ascontiguousarray(outt.T).reshape(4, 2048, EMB).astype(np.float32)
    if _trace:
        kernel.last_results = res
    return out


if __name__ == "__main__":
    rng = np.random.default_rng(0)
    x = rng.standard_normal((4, 2048, EMB), dtype=np.float32)
    w1 = (rng.standard_normal((HID, EMB), dtype=np.float32) * 0.03).astype(
        np.float32
    )
    w2 = (rng.standard_normal((HID, EMB), dtype=np.float32) * 0.03).astype(
        np.float32
    )
    w3 = (rng.standard_normal((EMB, HID), dtype=np.float32) * 0.015).astype(
        np.float32
    )
    out = kernel(x, w1, w2, w3)
    print("out", out.shape, out.dtype, float(np.abs(out).mean()))
